# revision 1
# baseline (speedup 1.0000x reference)
"""GQA multi-head attention (b=2, s=2048, d=2048, 32 Q heads / 8 KV heads,
head_dim=64, RoPE, causal) on 8 Trainium2 NeuronCores.

Sharding: tensor-parallel over heads x data-parallel over batch.
Core c = 4*bi + g handles batch bi and head-group g (8 Q heads, 2 KV heads).
Each core computes a partial [2048, 2048] output (its head block times the
matching wo rows); the host sums the 4 partials per batch.

Device kernel layout notes:
  - x arrives pre-transposed (xt = x[bi].T, [d, s]) so every matmul contracts
    over the partition dim with no on-device transposes of x.
  - q/k are produced in [head_dim, seq] ("transposed") layout, which is what
    both the scores matmul and the final wo matmul want as stationary.
  - scores are computed as S^T [sk, sq] so softmax's sum falls out of the
    ones-column trick in the PV matmul; no max-subtraction is needed because
    scores here are O(10).
  - Q heads are paired (m, m+4) per 128-partition m-tile so that the
    q-sub-block partition base always equals the kv head partition base
    (hardware requires equal base partitions for matmul operands).
"""

import sys

if "/opt/trn_rl_repo" not in sys.path:
    sys.path.insert(0, "/opt/trn_rl_repo")

import numpy as np

import concourse.bass as bass  # noqa: F401  (import keeps bass registered)
import concourse.tile as tile
from concourse import bacc, mybir
from concourse.bass_utils import run_bass_kernel_spmd

F32R = mybir.dt.float32r
F32 = mybir.dt.float32

S = 2048
D = 2048
NH = 32
NKV = 8
DH = 64
ROPE_BASE = 10000.0
N_CORES = 8
QH_PER_CORE = 8   # local q heads
KVH_PER_CORE = 2  # local kv heads
DQ = QH_PER_CORE * DH   # 512, per-core q width
DKV = KVH_PER_CORE * DH  # 128, per-core kv width

# module-level knobs the test harness can poke
RUN_KWARGS: dict = {}
LAST_RESULTS = None

_COMPILED = None


def _chunks_for(i):
    """sq chunks for sk-tile i: start at the diagonal (128*i), bank-aligned."""
    start = 128 * i
    out = []
    w = 512 - 128 * (i % 4)
    out.append((start, w))
    pos = start + w
    while pos < S:
        out.append((pos, 512))
        pos += 512
    return out


def _build(loop_n=1, phases=3, abl=0):
    nc = bacc.Bacc("TRN2", target_bir_lowering=False, debug=False)

    xt_d = nc.dram_tensor("xt", [D, S], F32R, kind="ExternalInput").ap()
    wall_d = nc.dram_tensor("wall", [128, 16 * 768], F32R, kind="ExternalInput").ap()
    wo_d = nc.dram_tensor("wo", [128, 4 * 2048], F32R, kind="ExternalInput").ap()
    cos_d = nc.dram_tensor("cos", [128, S], F32R, kind="ExternalInput").ap()
    sin_d = nc.dram_tensor("sin", [128, S], F32R, kind="ExternalInput").ap()
    tri_d = nc.dram_tensor("tri", [128, 128], F32R, kind="ExternalInput").ap()
    eye_d = nc.dram_tensor("eye", [128, 64], F32R, kind="ExternalInput").ap()
    out_d = nc.dram_tensor("out", [S, D], F32, kind="ExternalOutput").ap()

    import contextlib

    with tile.TileContext(nc) as tc:
        with (
            tc.For_i(0, loop_n, 1) if loop_n > 1 else contextlib.nullcontext()
        ):
            _phases(nc, tc, xt_d, wall_d, wo_d, cos_d, sin_d, tri_d, eye_d, out_d, phases, abl)

    nc.compile()
    return nc


def _phases(nc, tc, xt_d, wall_d, wo_d, cos_d, sin_d, tri_d, eye_d, out_d, phases=3, abl=0):
    if True:
        with tc.tile_pool(name="big", bufs=1) as big:
            qrot = big.tile([128, 4 * S], F32R)   # 4 m-tiles of [2 heads x 64, S]
            krot = big.tile([128, S], F32R)       # [2 kv heads x 64, S]
            vaug = big.tile([128, 2 * 16 * 65], F32R)  # per (kv, sk-tile): [128, 65]
            attn = big.tile([128, 4 * S], F32R)   # normalized attention, qrot layout
            tri_sb = big.tile([128, 128], F32R)
            nc.sync.dma_start(tri_sb[:], tri_d[:])

            # ---------------- phase 1: projections + rope + v transpose -----
            with (
                tc.tile_pool(name="p1", bufs=1) as p1,
                tc.tile_pool(name="p1s", bufs=6) as p1s,
                tc.tile_pool(name="rope", bufs=2) as ropep,
                tc.tile_pool(name="ps1", bufs=1, space="PSUM") as ps1,
                tc.tile_pool(name="ps1t", bufs=2, space="PSUM") as ps1t,
            ):
                w_sb = p1.tile([128, 16 * 768], F32R)
                for m in range(6):
                    nc.sync.dma_start(
                        w_sb[:, m * 2048 : (m + 1) * 2048],
                        wall_d[:, m * 2048 : (m + 1) * 2048],
                    )
                cos_sb = p1.tile([128, S], F32R)
                nc.sync.dma_start(cos_sb[:], cos_d[:])
                sin_sb = p1.tile([128, S], F32R)
                nc.sync.dma_start(sin_sb[:], sin_d[:])
                eye_sb = p1.tile([128, 64], F32R)
                nc.sync.dma_start(eye_sb[:], eye_d[:])
                vt_sb = p1.tile([128, S], F32R)
                onecol = p1.tile([128, 1], F32)
                nc.vector.memset(onecol[:], 1.0)

                def rope_evac(dst, ps, sc, w):
                    # dst = ps * cos + shift32(ps * sin); all within [128, w]
                    cs = cos_sb[:, sc * 512 : sc * 512 + w]
                    sn = sin_sb[:, sc * 512 : sc * 512 + w]
                    m1 = ropep.tile([128, 512], F32R, tag="m1")
                    m2 = ropep.tile([128, 512], F32R, tag="m2")
                    nc.vector.tensor_tensor(
                        m1[:, :w], ps[:, :w], cs, op=mybir.AluOpType.mult
                    )
                    # write the sin product pre-shifted (partition p^32)
                    for q in range(4):
                        a, b = q * 32, (q ^ 1) * 32
                        nc.vector.tensor_tensor(
                            m2[b : b + 32, :w],
                            ps[a : a + 32, :w],
                            sn[a : a + 32, :],
                            op=mybir.AluOpType.mult,
                        )
                    nc.vector.tensor_tensor(
                        dst, m1[:, :w], m2[:, :w], op=mybir.AluOpType.add
                    )

                for sc in range(4):
                    pss = [
                        ps1.tile([128, 512], F32, tag=f"proj{m}", name=f"pj{m}_{sc}")
                        for m in range(6)
                    ]
                    for k in range(16):
                        xt_t = p1s.tile([128, 512], F32R, tag="xt", name=f"xt{sc}_{k}")
                        nc.sync.dma_start(
                            xt_t[:],
                            xt_d[k * 128 : (k + 1) * 128, sc * 512 : (sc + 1) * 512],
                        )
                        for m in range(6):
                            nc.tensor.matmul(
                                pss[m][:],
                                lhsT=w_sb[:, m * 2048 + k * 128 : m * 2048 + (k + 1) * 128],
                                rhs=xt_t[:],
                                start=(k == 0),
                                stop=(k == 15),
                            )
                    for m in range(4):
                        rope_evac(
                            qrot[:, m * S + sc * 512 : m * S + sc * 512 + 512],
                            pss[m], sc, 512,
                        )
                    rope_evac(krot[:, sc * 512 : sc * 512 + 512], pss[4], sc, 512)
                    nc.scalar.copy(vt_sb[:, sc * 512 : (sc + 1) * 512], pss[5][:])

                # v: [2 kv x 64, S] -> natural [sk, 64] blocks + ones column
                for kv in range(2):
                    for i in range(16):
                        base = (kv * 16 + i) * 65
                        tp = ps1t.tile([128, 64], F32R, tag="vtp", name=f"vtp{kv}_{i}")
                        nc.tensor.transpose(
                            tp[:],
                            vt_sb[kv * 64 : (kv + 1) * 64, i * 128 : (i + 1) * 128],
                            eye_sb[kv * 64 : (kv + 1) * 64, :],
                        )
                        nc.vector.tensor_copy(vaug[:, base : base + 64], tp[:])
                        nc.vector.tensor_copy(
                            vaug[:, base + 64 : base + 65], onecol[:]
                        )

            if phases < 2:
                # timing-only: flush qrot so phase 1 has a consumer
                with tc.tile_pool(name="tf", bufs=2) as tf:
                    for mm in range(4):
                        otf = tf.tile([128, 2048], F32, tag="otf", name=f"otf{mm}")
                        nc.scalar.copy(otf[:], qrot[:, mm * S : (mm + 1) * S])
                        nc.sync.dma_start(out_d[mm * 128 : (mm + 1) * 128, :], otf[:])
                return

            # ---------------- phase 2: attention --------------------------
            with tc.tile_pool(name="p2", bufs=1) as p2:
                wo_sb = p2.tile([128, 4 * 2048], F32R)
                nc.sync.dma_start(wo_sb[:], wo_d[:])

                with (
                    tc.tile_pool(name="probs", bufs=4) as probsp,
                    tc.tile_pool(name="recp", bufs=2) as recp,
                    tc.tile_pool(name="ps2o", bufs=2, space="PSUM") as ps2o,
                    tc.tile_pool(name="ps2s", bufs=3, space="PSUM") as ps2s,
                ):
                    for c in range(4):
                        for m in range(4):
                            for sub in range(2):
                                hb = sub * 64  # q base == kv base
                                q_chunk = qrot[hb : hb + 64, m * S + c * 512 : m * S + (c + 1) * 512]
                                out_ps = ps2o.tile(
                                    [65, 512], F32, tag="outps", name=f"ops{m}_{sub}_{c}"
                                )
                                n_i = 4 * c + 4
                                for g in range(0, n_i, 2):
                                    cnt = min(2, n_i - g)
                                    sc_ps = ps2s.tile(
                                        [128, 1024], F32, tag="scps",
                                        name=f"sc{m}_{sub}_{c}_{g}",
                                    )
                                    for j in range(cnt):
                                        i = g + j
                                        nc.tensor.matmul(
                                            sc_ps[:, j * 512 : (j + 1) * 512],
                                            lhsT=krot[hb : hb + 64, i * 128 : (i + 1) * 128],
                                            rhs=q_chunk,
                                            start=True,
                                            stop=True,
                                        )
                                    if abl == 1:
                                        continue
                                    pr = probsp.tile(
                                        [128, 1024], F32R, tag="pr",
                                        name=f"pr{m}_{sub}_{c}_{g}",
                                    )
                                    nc.scalar.activation(
                                        pr[:, : cnt * 512],
                                        sc_ps[:, : cnt * 512],
                                        mybir.ActivationFunctionType.Exp,
                                        scale=0.125,
                                    )
                                    if abl == 2:
                                        continue
                                    for j in range(cnt):
                                        i = g + j
                                        r = i - 4 * c
                                        if r >= 0:  # diagonal band: triangular mask
                                            lo = j * 512 + 128 * r
                                            nc.vector.tensor_tensor(
                                                pr[:, lo : lo + 128],
                                                pr[:, lo : lo + 128],
                                                tri_sb[:],
                                                op=mybir.AluOpType.mult,
                                            )
                                        off = max(0, 128 * r)
                                        nc.tensor.matmul(
                                            out_ps[:, off:512],
                                            lhsT=vaug[:, (sub * 16 + i) * 65 : (sub * 16 + i) * 65 + 65],
                                            rhs=pr[:, j * 512 + off : (j + 1) * 512],
                                            start=(i == 0),
                                            stop=(i == n_i - 1),
                                            skip_group_check=True,
                                        )
                                if abl:
                                    src_ap = sc_ps[0:64, 0:512] if abl == 1 else pr[0:64, 0:512]
                                    nc.vector.tensor_copy(
                                        attn[hb : hb + 64, m * S + c * 512 : m * S + (c + 1) * 512],
                                        src_ap,
                                    )
                                    continue
                                # normalize: attn chunk = out_ps[0:64] / denom row
                                recip = recp.tile(
                                    [1, 512], F32, tag="recip", name=f"rc{m}_{sub}_{c}"
                                )
                                nc.vector.reciprocal(recip[:], out_ps[64:65, :])
                                rec64 = recp.tile(
                                    [64, 512], F32, tag="rec64", name=f"rb{m}_{sub}_{c}"
                                )
                                nc.gpsimd.partition_broadcast(rec64[:], recip[:])
                                nc.vector.tensor_tensor(
                                    attn[hb : hb + 64, m * S + c * 512 : m * S + (c + 1) * 512],
                                    out_ps[0:64, :],
                                    rec64[:],
                                    op=mybir.AluOpType.mult,
                                )

                if phases < 3:
                    with tc.tile_pool(name="tf3", bufs=2) as tf3:
                        for mm in range(4):
                            otf = tf3.tile([128, 2048], F32, tag="otf3", name=f"o3f{mm}")
                            nc.scalar.copy(otf[:], attn[:, mm * S : (mm + 1) * S])
                            nc.sync.dma_start(out_d[mm * 128 : (mm + 1) * 128, :], otf[:])
                    return

                # ---------------- phase 3: output projection ----------------
                with (
                    tc.tile_pool(name="p3", bufs=4) as p3,
                    tc.tile_pool(name="ps3", bufs=4, space="PSUM") as ps3,
                ):
                    for st in range(16):
                        for nk in range(4):
                            ps = ps3.tile(
                                [128, 512], F32, tag="wops", name=f"wo{st}_{nk}"
                            )
                            for kt in range(4):
                                nc.tensor.matmul(
                                    ps[:],
                                    lhsT=attn[:, kt * S + st * 128 : kt * S + st * 128 + 128],
                                    rhs=wo_sb[:, kt * 2048 + nk * 512 : kt * 2048 + (nk + 1) * 512],
                                    start=(kt == 0),
                                    stop=(kt == 3),
                                )
                            ot = p3.tile([128, 512], F32, tag="ot", name=f"ot{st}_{nk}")
                            nc.scalar.copy(ot[:], ps[:])
                            nc.sync.dma_start(
                                out_d[st * 128 : (st + 1) * 128, nk * 512 : (nk + 1) * 512],
                                ot[:],
                            )


def _get_compiled():
    global _COMPILED
    if _COMPILED is None:
        _COMPILED = _build()
    return _COMPILED


def _host_tables():
    invf = ROPE_BASE ** (-np.arange(0, DH, 2, dtype=np.float64) / DH)  # [32]
    t = np.arange(S, dtype=np.float64)
    theta = t[None, :] * invf[:, None]  # [32, S]
    c32 = np.cos(theta)
    s32 = np.sin(theta)
    C = np.empty((128, S), np.float32)
    Sg = np.empty((128, S), np.float32)
    for j in range(2):
        C[j * 64 : j * 64 + 32] = c32
        C[j * 64 + 32 : j * 64 + 64] = c32
        Sg[j * 64 : j * 64 + 32] = s32          # +sin for first half
        Sg[j * 64 + 32 : j * 64 + 64] = -s32    # -sin for second half
    tri = np.triu(np.ones((128, 128), np.float32))  # tri[a,b]=1 iff a<=b
    eye = np.tile(np.eye(64, dtype=np.float32), (2, 1))
    return C, Sg, tri, eye


# device head order within the 512-wide q shard: m-tile m holds local heads
# (m, m+4) so that the q sub-block base (64*sub) equals the kv base (64*kv).
_PERM_Q = np.array(
    [(m + 4 * sub) * DH + d for m in range(4) for sub in range(2) for d in range(DH)],
    dtype=np.int64,
)


def _rearrange_w(w):  # [2048, 768] -> [128, 12288] m-tile-major
    # device slice for (m, k) is w_dev[:, m*2048 + k*128 : +128]
    return np.ascontiguousarray(
        w.reshape(16, 128, 6, 128).transpose(1, 2, 0, 3).reshape(128, 16 * 768)
    )


def _rearrange_wo(w):  # [512, 2048] -> [128, 8192]
    return np.ascontiguousarray(
        w.reshape(4, 128, 2048).transpose(1, 0, 2).reshape(128, 4 * 2048)
    )


def _make_in_maps(ins):
    x = np.asarray(ins["x"], np.float32)
    wq = np.asarray(ins["wq"], np.float32)
    wk = np.asarray(ins["wk"], np.float32)
    wv = np.asarray(ins["wv"], np.float32)
    wo = np.asarray(ins["wo"], np.float32)

    C, Sg, tri, eye = _host_tables()
    xts = [np.ascontiguousarray(x[bi].T) for bi in range(2)]

    in_maps = []
    for c in range(N_CORES):
        bi, g = c // 4, c % 4
        wq_s = wq[:, g * DQ : (g + 1) * DQ][:, _PERM_Q]
        wk_s = wk[:, g * DKV : (g + 1) * DKV]
        wv_s = wv[:, g * DKV : (g + 1) * DKV]
        wall = _rearrange_w(
            np.ascontiguousarray(np.concatenate([wq_s, wk_s, wv_s], axis=1))
        )
        wo_s = _rearrange_wo(np.ascontiguousarray(wo[g * DQ : (g + 1) * DQ, :][_PERM_Q]))
        in_maps.append(
            {
                "xt": xts[bi],
                "wall": wall,
                "wo": wo_s,
                "cos": C,
                "sin": Sg,
                "tri": tri,
                "eye": eye,
            }
        )
    return in_maps


def kernel(x, wq, wk, wv, wo):
    global LAST_RESULTS
    nc = _get_compiled()
    in_maps = _make_in_maps({"x": x, "wq": wq, "wk": wk, "wv": wv, "wo": wo})
    res = run_bass_kernel_spmd(nc, in_maps, list(range(N_CORES)), **RUN_KWARGS)
    LAST_RESULTS = res
    out = np.empty((2, S, D), np.float32)
    for bi in range(2):
        acc = res.results[4 * bi]["out"].astype(np.float32)
        for g in range(1, 4):
            acc = acc + res.results[4 * bi + g]["out"]
        out[bi] = acc
    return out

